# revision 1
# baseline (speedup 1.0000x reference)
"""MoE ConditionalFeedForward (SwiGLU, T=2048 D=1024 I=4096 E=8 K=2) on 8 TRN2 cores.

Strategy: expert-parallel, one expert per NeuronCore. Routing/gather happens on
host (numpy): for each expert e, collect the unique tokens routed to it, merge
the two top-k gate weights, and ship the gathered tokens transposed plus that
expert's three weight matrices, pre-packed so every device DMA is a fully
linear HBM read. Each core computes
  y_e = (silu(x @ w1e^T) * (x @ w3e^T)) @ w2e^T * gate
for its <=CAP tokens; the host scatter-adds the 8 partials into [T, D].

Device kernel (per core): all matmuls in float32r (fp32 operands truncated to
FP22 in the PE -> ~full rate at N=512, ~2e-4 rel err).
  layer 1: per i-tile, accumulate 8 K=128 steps into two PSUM banks (h1, h3),
           then ACT silu + DVE multiply into an SBUF hT tile laid out [i, t]
           so it feeds layer 2 as lhsT directly.
  layer 2: y[t, d] accumulated over 32 i-tiles into 8 PSUM banks
           (4 t-tiles x 2 d-chunks); gate applied as a per-partition scale on
           the PSUM->SBUF copy (alternating ACT/DVE to shorten the tail).
"""

import math
import os
import sys
import time
import types

for _p in ("/opt/trn_rl_repo", "/opt/pypackages"):
    if _p not in sys.path:
        sys.path.append(_p)

import numpy as np

# antenv.axon_hooks is absent from this image; run_bass_kernel_spmd imports it
# unconditionally when tracing is requested (BASS_TRACE=1). Provide the
# documented shim so profiling works when asked for and degrades to a no-op
# otherwise. No-op if a real antenv.axon_hooks exists.
def _ensure_ntff_hook():
    try:
        import antenv
    except ImportError:
        return
    try:
        import antenv.axon_hooks  # noqa: F401
        return
    except ImportError:
        pass
    mod = types.ModuleType("antenv.axon_hooks")
    mod._hook = None

    def set_axon_ntff_profile_hook(h):
        mod._hook = h

    def get_axon_ntff_profile_hook():
        if mod._hook is None:
            try:
                from trn_agent_boot.trn_boot import _ntff_profile_via_ctypes

                mod._hook = _ntff_profile_via_ctypes("/opt/axon/libaxon_pjrt.so")
            except Exception:
                mod._hook = None
        return mod._hook

    mod.set_axon_ntff_profile_hook = set_axon_ntff_profile_hook
    mod.get_axon_ntff_profile_hook = get_axon_ntff_profile_hook
    sys.modules["antenv.axon_hooks"] = mod
    antenv.axon_hooks = mod


_ensure_ntff_hook()

import concourse.bacc as bacc
import concourse.tile as tile
from concourse import mybir
from concourse.bass_utils import run_bass_kernel_spmd

T, D, I, E, TOPK = 2048, 1024, 4096, 8, 2
N_CORES = 8
CAP = 512            # tokens per expert per pass (multiple of 128, <=512)
NT = CAP // 128      # 4 token tiles
DT = D // 128        # 8 contraction steps for layer 1
NI = I // 128        # 32 intermediate tiles
F32 = mybir.dt.float32
F32R = mybir.dt.float32r

_NC = None           # compiled Bass module, built once per process
_WCACHE = {}         # packed per-expert weights, keyed on input identity
LAST_RESULTS = None  # BassKernelResults of the most recent SPMD run


def _build_nc(sim_act=False):
    # sim_act: CoreSim lacks Silu; emit sigmoid + extra multiply instead
    # (same math) so the program can be validated in simulation.
    nc = bacc.Bacc(
        "TRN2", target_bir_lowering=False, debug=False, num_devices=N_CORES
    )
    # Packed layouts (see _pack_weights): every DMA below reads HBM linearly.
    xt_d = nc.dram_tensor("xt", [DT, 128, CAP], F32R, kind="ExternalInput").ap()
    g_d = nc.dram_tensor("g", [CAP], F32, kind="ExternalInput").ap()
    w13p_d = nc.dram_tensor(
        "w13p", [NI, 2, 128, DT, 128], F32R, kind="ExternalInput"
    ).ap()
    w2t_d = nc.dram_tensor("w2t", [I, D], F32R, kind="ExternalInput").ap()
    y_d = nc.dram_tensor("y", [CAP, D], F32, kind="ExternalOutput").ap()

    with tile.TileContext(nc) as tc:
        with (
            tc.tile_pool(name="consts", bufs=1) as const_pool,
            tc.tile_pool(name="w13", bufs=7) as w13_pool,
            tc.tile_pool(name="w2", bufs=7) as w2_pool,
            tc.tile_pool(name="h", bufs=1) as h_pool,
            tc.tile_pool(name="tmp", bufs=2) as tmp_pool,
            tc.tile_pool(name="yout", bufs=4) as out_pool,
        ):
            # Resident activations: x^T as 8 [128, CAP] d-tiles (one DMA per
            # d-tile so the first matmul is gated on 256 KB, not 2 MB), gates.
            xt_sb = const_pool.tile([128, DT, CAP], F32R)
            for dt_i in range(DT):
                if dt_i == 0:
                    # Halve the startup-critical slice: first matmul gates on
                    # 128 KB arriving on two queues in parallel.
                    nc.sync.dma_start(xt_sb[:, 0, :CAP // 2], xt_d[0][:, :CAP // 2])
                    nc.sync.dma_start(xt_sb[:, 0, CAP // 2:], xt_d[0][:, CAP // 2:])
                else:
                    nc.sync.dma_start(xt_sb[:, dt_i, :], xt_d[dt_i])
            g_sb = const_pool.tile([128, NT], F32)
            nc.sync.dma_start(g_sb[:], g_d.rearrange("(a p) -> p a", p=128))

            # hT[i, t] — layer-1 output, transposed so it is lhsT for layer 2.
            hT = h_pool.tile([128, NI, CAP], F32R)

            ps1_pool = tc.alloc_tile_pool(name="ps1", bufs=2, space="PSUM")
            for it in range(NI):
                w13_t = w13_pool.tile([128, 2, DT, 128], F32R, tag="w13")
                w1_t = w13_t[:, 0]
                w3_t = w13_t[:, 1]
                if it == 0:
                    # Startup-critical loads go through GPSIMD's SWDGE queues,
                    # in parallel with the xt loads saturating the HWDGE
                    # queues, split so matmul dt_i waits only on its 64 KB.
                    for m in range(2):
                        for dt_i in range(DT):
                            nc.gpsimd.dma_start(
                                w13_t[:, m, dt_i, :], w13p_d[0, m, :, dt_i, :]
                            )
                elif it <= 2:
                    # Ramp-critical tiles: quarter the load across queues so
                    # per-queue latency (~11us for 1 MB) doesn't starve the PE.
                    for m in range(2):
                        for h in range(2):
                            lo = h * (DT // 2)
                            nc.sync.dma_start(
                                w13_t[:, m, lo:lo + DT // 2, :],
                                w13p_d[it, m][:, lo:lo + DT // 2, :],
                            )
                elif it <= 5:
                    for m in range(2):
                        nc.sync.dma_start(w13_t[:, m], w13p_d[it, m])
                else:
                    # One 1 MB linear DMA per i-tile (fewer issues, fewer sems).
                    nc.sync.dma_start(
                        w13_t[:], w13p_d[it].rearrange("m p a c -> p m a c")
                    )
                h1_ps = ps1_pool.tile([128, CAP], F32, tag="h1")
                h3_ps = ps1_pool.tile([128, CAP], F32, tag="h3")
                for dt_i in range(DT):
                    nc.tensor.matmul(
                        h1_ps[:],
                        w1_t[:, dt_i, :],
                        xt_sb[:, dt_i, :],
                        start=(dt_i == 0),
                        stop=(dt_i == DT - 1),
                    )
                for dt_i in range(DT):
                    nc.tensor.matmul(
                        h3_ps[:],
                        w3_t[:, dt_i, :],
                        xt_sb[:, dt_i, :],
                        start=(dt_i == 0),
                        stop=(dt_i == DT - 1),
                    )
                s_sb = tmp_pool.tile([128, CAP], F32)
                if sim_act:
                    nc.scalar.activation(
                        s_sb[:], h1_ps[:], mybir.ActivationFunctionType.Sigmoid
                    )
                    nc.vector.tensor_mul(s_sb[:], s_sb[:], h1_ps[:])
                else:
                    nc.scalar.activation(
                        s_sb[:], h1_ps[:], mybir.ActivationFunctionType.Silu
                    )
                nc.vector.tensor_mul(hT[:, it, :], s_sb[:], h3_ps[:])

            ps1_pool.release()

            # Layer 2: y[t, d] over 8 PSUM banks, accumulated across i-tiles.
            ps2_pool = tc.alloc_tile_pool(name="ps2", bufs=1, space="PSUM")
            y_ps = ps2_pool.tile([128, NT * 2, 512], F32)
            w2t_r = w2t_d.rearrange("(a p) d -> p a d", p=128)
            for it in range(NI):
                w2_t = w2_pool.tile([128, D], F32R)
                nc.sync.dma_start(w2_t[:], w2t_r[:, it, :])
                for tt in range(NT):
                    for dc in range(2):
                        nc.tensor.matmul(
                            y_ps[:, tt * 2 + dc, :],
                            hT[:, it, tt * 128:(tt + 1) * 128],
                            w2_t[:, dc * 512:(dc + 1) * 512],
                            start=(it == 0),
                            stop=(it == NI - 1),
                        )
            # Gate-scaled PSUM->SBUF copies split across ACT and DVE so the
            # kernel tail drains in parallel; each 256 KB half DMAs out as
            # soon as its copy lands so the final transfers overlap the
            # remaining copies across queues.
            for tt in range(NT):
                y_sb = out_pool.tile([128, D], F32)
                for dc in range(2):
                    src = y_ps[:, tt * 2 + dc, :]
                    dst = y_sb[:, dc * 512:(dc + 1) * 512]
                    if dc == 0:
                        nc.scalar.activation(
                            dst, src, mybir.ActivationFunctionType.Copy,
                            scale=g_sb[:, tt:tt + 1],
                        )
                    else:
                        nc.vector.tensor_scalar_mul(dst, src, g_sb[:, tt:tt + 1])
                    if tt == NT - 1:
                        for q in range(2):
                            lo = dc * 512 + q * 256
                            nc.sync.dma_start(
                                y_d[tt * 128:(tt + 1) * 128, lo:lo + 256],
                                y_sb[:, lo:lo + 256],
                            )
                    else:
                        nc.sync.dma_start(
                            y_d[tt * 128:(tt + 1) * 128, dc * 512:(dc + 1) * 512],
                            dst,
                        )
            ps2_pool.release()

    nc.compile()
    return nc


def _pack_weights(w1, w2, w3):
    """Per-expert device layouts, all linear HBM reads:
    w1p/w3p[it, p, dt, c] = w[it*128+c, dt*128+p]  (i.e. w.T tiled for lhsT)
    w2t = w2.T ([I, D], i rows on partitions)."""
    key = tuple((a.ctypes.data, a.shape) for a in (w1, w2, w3))
    if _WCACHE.get("key") == key:
        return _WCACHE["maps"]
    maps = []
    for e in range(E):
        w13p = np.empty((NI, 2, 128, DT, 128), dtype=np.float32)
        w13p[:, 0] = w1[e].reshape(NI, 128, DT, 128).transpose(0, 3, 2, 1)
        w13p[:, 1] = w3[e].reshape(NI, 128, DT, 128).transpose(0, 3, 2, 1)
        w2t = np.ascontiguousarray(w2[e].T)
        maps.append({"w13p": w13p, "w2t": w2t})
    _WCACHE["key"] = key
    _WCACHE["maps"] = maps
    return maps


def kernel(x, expert_indices, expert_weights, w1, w2, w3):
    global _NC, LAST_RESULTS
    x = np.ascontiguousarray(np.asarray(x, dtype=np.float32))
    idx = np.asarray(expert_indices)
    ew = np.asarray(expert_weights, dtype=np.float32)
    w1 = np.ascontiguousarray(np.asarray(w1, dtype=np.float32))
    w2 = np.ascontiguousarray(np.asarray(w2, dtype=np.float32))
    w3 = np.ascontiguousarray(np.asarray(w3, dtype=np.float32))

    if _NC is None:
        _NC = _build_nc()

    # Host routing: unique tokens per expert, with both top-k gate weights of a
    # token merged (a token picking the same expert twice gets the summed gate).
    tok_lists, gate_lists = [], []
    for e in range(E):
        m = idx == e
        sel = np.nonzero(m.any(axis=1))[0]
        tok_lists.append(sel)
        gate_lists.append((ew * m).sum(axis=1)[sel].astype(np.float32))

    weight_maps = _pack_weights(w1, w2, w3)

    n_pass = max(1, math.ceil(max(len(s) for s in tok_lists) / CAP))
    out = np.zeros((T, D), dtype=np.float32)
    trace = bool(os.environ.get("BASS_TRACE"))
    for p in range(n_pass):
        in_maps = []
        chunks = []
        for e in range(E):
            sel = tok_lists[e][p * CAP:(p + 1) * CAP]
            g = gate_lists[e][p * CAP:(p + 1) * CAP]
            chunks.append(sel)
            xt = np.zeros((DT, 128, CAP), dtype=np.float32)
            if len(sel):
                xt.reshape(D, CAP)[:, :len(sel)] = x[sel].T
            g_pad = np.zeros((CAP,), dtype=np.float32)
            g_pad[:len(sel)] = g
            in_maps.append({"xt": xt, "g": g_pad, **weight_maps[e]})
        # Rare transient NRT_EXEC_UNIT_UNRECOVERABLE errors have been observed
        # on the first execution of a fresh NEFF; a straight retry recovers.
        last_exc = None
        for attempt in range(3):
            try:
                LAST_RESULTS = run_bass_kernel_spmd(
                    _NC, in_maps, core_ids=list(range(N_CORES)),
                    trace=trace and attempt == 0,
                )
                break
            except Exception as exc:  # noqa: BLE001
                last_exc = exc
                time.sleep(3)
        else:
            raise last_exc
        for e in range(E):
            sel = chunks[e]
            if len(sel):
                out[sel] += LAST_RESULTS.results[e]["y"][:len(sel)]
    return out



# revision 2
# speedup vs baseline: 1.1087x; 1.1087x over previous
"""MoE ConditionalFeedForward (SwiGLU, T=2048 D=1024 I=4096 E=8 K=2) on 8 TRN2 cores.

Strategy: expert-parallel, one expert per NeuronCore. Routing/gather happens on
host (numpy): for each expert e, collect the unique tokens routed to it, merge
the two top-k gate weights, and ship the gathered tokens transposed plus that
expert's three weight matrices, pre-packed so every device DMA is a fully
linear HBM read. Each core computes
  y_e = (silu(x @ w1e^T) * (x @ w3e^T)) @ w2e^T * gate
for its <=CAP tokens; the host scatter-adds the 8 partials into [T, D].

All matmul operands are bf16 (PSUM accumulation stays fp32): same PE rate as
float32r but half the HBM traffic, so the DMA stream never gates the PE and the
startup x/w loads land in half the time. End-to-end absmax rel err ~4e-3.

Device kernel (per core):
  warmup: a dozen matmuls on a zeroed scratch tile, issued before any
          DMA-dependent work, burn the PE DVFS ramp (0.65/1.2 GHz pstates for
          the first ~3us of busy) while the first x/w tiles stream in.
  layer 1: per i-tile, accumulate 8 K=128 steps into two PSUM banks (h1, h3),
           then ACT silu + DVE multiply into an SBUF hT tile laid out [i, t]
           (bf16) so it feeds layer 2 as lhsT directly.
  layer 2: w2 is fully SBUF-resident (8 MB bf16, prefetched during layer 1);
           t-outer loop so each 128-token tile's two PSUM banks accumulate all
           32 i-steps back to back, then drain (gate applied as a per-partition
           scale on the PSUM->SBUF copy, ACT for one bank / DVE for the other)
           and DMA out while the next tile accumulates. Only the last 256 KB
           drain remains in the kernel tail.
"""

import math
import os
import sys
import time
import types

for _p in ("/opt/trn_rl_repo", "/opt/pypackages"):
    if _p not in sys.path:
        sys.path.append(_p)

import ml_dtypes
import numpy as np

# antenv.axon_hooks is absent from this image; run_bass_kernel_spmd imports it
# unconditionally when tracing is requested (BASS_TRACE=1). Provide the
# documented shim so profiling works when asked for and degrades to a no-op
# otherwise. No-op if a real antenv.axon_hooks exists.
def _ensure_ntff_hook():
    try:
        import antenv
    except ImportError:
        return
    try:
        import antenv.axon_hooks  # noqa: F401
        return
    except ImportError:
        pass
    mod = types.ModuleType("antenv.axon_hooks")
    mod._hook = None

    def set_axon_ntff_profile_hook(h):
        mod._hook = h

    def get_axon_ntff_profile_hook():
        if mod._hook is None:
            try:
                from trn_agent_boot.trn_boot import _ntff_profile_via_ctypes

                mod._hook = _ntff_profile_via_ctypes("/opt/axon/libaxon_pjrt.so")
            except Exception:
                mod._hook = None
        return mod._hook

    mod.set_axon_ntff_profile_hook = set_axon_ntff_profile_hook
    mod.get_axon_ntff_profile_hook = get_axon_ntff_profile_hook
    sys.modules["antenv.axon_hooks"] = mod
    antenv.axon_hooks = mod


_ensure_ntff_hook()

import concourse.bacc as bacc
import concourse.tile as tile
from concourse import mybir
from concourse.bass_utils import run_bass_kernel_spmd

T, D, I, E, TOPK = 2048, 1024, 4096, 8, 2
N_CORES = 8
CAP = 512            # tokens per expert per pass (multiple of 128, <=512)
NT = CAP // 128      # 4 token tiles
DT = D // 128        # 8 contraction steps for layer 1
NI = I // 128        # 32 intermediate tiles
N_WARM = 12          # PE warmup matmuls (~4.5us: DVFS ramp + startup DMA window)
F32 = mybir.dt.float32
BF16 = mybir.dt.bfloat16
BF = ml_dtypes.bfloat16

_NC = None           # compiled Bass module, built once per process
_WCACHE = {}         # packed per-expert weights, keyed on input identity
LAST_RESULTS = None  # BassKernelResults of the most recent SPMD run


def _build_nc(sim_act=False):
    # sim_act: CoreSim lacks Silu; emit sigmoid + extra multiply instead
    # (same math) so the program can be validated in simulation.
    nc = bacc.Bacc(
        "TRN2", target_bir_lowering=False, debug=False, num_devices=N_CORES
    )
    # Packed layouts (see _pack_weights): every DMA below reads HBM linearly.
    xt_d = nc.dram_tensor("xt", [DT, 128, CAP], BF16, kind="ExternalInput").ap()
    g_d = nc.dram_tensor("g", [CAP], F32, kind="ExternalInput").ap()
    w13p_d = nc.dram_tensor(
        "w13p", [NI, 2, 128, DT, 128], BF16, kind="ExternalInput"
    ).ap()
    w2t_d = nc.dram_tensor("w2t", [I, D], BF16, kind="ExternalInput").ap()
    y_d = nc.dram_tensor("y", [CAP, D], F32, kind="ExternalOutput").ap()

    with tile.TileContext(nc) as tc:
        with (
            tc.tile_pool(name="consts", bufs=1) as const_pool,
            tc.tile_pool(name="w13", bufs=6) as w13_pool,
            tc.tile_pool(name="h", bufs=1) as h_pool,
            tc.tile_pool(name="tmp", bufs=2) as tmp_pool,
            tc.tile_pool(name="yout", bufs=4) as out_pool,
        ):
            # PE warmup: no DMA dependencies, so these issue immediately and
            # carry the PE through its 0.65/1.2 GHz DVFS pstates while the
            # first real tiles stream in. Results are never read.
            ws = const_pool.tile([128, 512], BF16)
            nc.vector.memset(ws[:], 0.0)
            psw_pool = tc.alloc_tile_pool(name="psw", bufs=1, space="PSUM")
            warm_ps = psw_pool.tile([128, 512], F32)
            for k in range(N_WARM):
                nc.tensor.matmul(
                    warm_ps[:], ws[:, :128], ws[:],
                    start=(k == 0), stop=(k == N_WARM - 1),
                )

            # Resident activations: x^T as 8 [128, CAP] d-tiles (one DMA per
            # d-tile; the first is split so the opening matmul gates on two
            # 64 KB transfers landing on parallel queues), gates.
            xt_sb = const_pool.tile([128, DT, CAP], BF16)
            for dt_i in range(DT):
                if dt_i == 0:
                    nc.sync.dma_start(xt_sb[:, 0, :CAP // 2], xt_d[0][:, :CAP // 2])
                    nc.sync.dma_start(xt_sb[:, 0, CAP // 2:], xt_d[0][:, CAP // 2:])
                else:
                    nc.sync.dma_start(xt_sb[:, dt_i, :], xt_d[dt_i])
            g_sb = const_pool.tile([128, NT], F32)
            nc.sync.dma_start(g_sb[:], g_d.rearrange("(a p) -> p a", p=128))

            # w2 lives in SBUF for all of layer 2 (64 KB/partition bf16);
            # i-tile loads are spread across the layer-1 iterations below so
            # they never contend with the startup-critical x/w13 transfers.
            w2_sb = const_pool.tile([128, NI, D], BF16)
            w2t_r = w2t_d.rearrange("(a p) d -> p a d", p=128)

            # hT[i, t] — layer-1 output (bf16), transposed so it is lhsT for
            # layer 2.
            hT = h_pool.tile([128, NI, CAP], BF16)

            ps1_pool = tc.alloc_tile_pool(name="ps1", bufs=2, space="PSUM")
            for it in range(NI):
                w13_t = w13_pool.tile([128, 2, DT, 128], BF16, tag="w13")
                w1_t = w13_t[:, 0]
                w3_t = w13_t[:, 1]
                if it == 0:
                    # Startup-critical loads go through GPSIMD's SWDGE queues,
                    # in parallel with the xt loads saturating the HWDGE
                    # queues, split so matmul dt_i waits only on its 32 KB.
                    for m in range(2):
                        for dt_i in range(DT):
                            nc.gpsimd.dma_start(
                                w13_t[:, m, dt_i, :], w13p_d[0, m, :, dt_i, :]
                            )
                elif it <= 2:
                    # Ramp-critical tiles: halve the load across queues so
                    # per-queue latency doesn't starve the PE.
                    for m in range(2):
                        for h in range(2):
                            lo = h * (DT // 2)
                            nc.sync.dma_start(
                                w13_t[:, m, lo:lo + DT // 2, :],
                                w13p_d[it, m][:, lo:lo + DT // 2, :],
                            )
                else:
                    # One 512 KB linear DMA per i-tile (fewer issues/sems).
                    nc.sync.dma_start(
                        w13_t[:], w13p_d[it].rearrange("m p a c -> p m a c")
                    )
                # Prefetch w2 i-tiles once the startup burst has drained.
                if it >= 4:
                    nc.sync.dma_start(w2_sb[:, it - 4, :], w2t_r[:, it - 4, :])
                h1_ps = ps1_pool.tile([128, CAP], F32, tag="h1")
                h3_ps = ps1_pool.tile([128, CAP], F32, tag="h3")
                for dt_i in range(DT):
                    nc.tensor.matmul(
                        h1_ps[:],
                        w1_t[:, dt_i, :],
                        xt_sb[:, dt_i, :],
                        start=(dt_i == 0),
                        stop=(dt_i == DT - 1),
                    )
                for dt_i in range(DT):
                    nc.tensor.matmul(
                        h3_ps[:],
                        w3_t[:, dt_i, :],
                        xt_sb[:, dt_i, :],
                        start=(dt_i == 0),
                        stop=(dt_i == DT - 1),
                    )
                s_sb = tmp_pool.tile([128, CAP], F32)
                if sim_act:
                    nc.scalar.activation(
                        s_sb[:], h1_ps[:], mybir.ActivationFunctionType.Sigmoid
                    )
                    nc.vector.tensor_mul(s_sb[:], s_sb[:], h1_ps[:])
                else:
                    nc.scalar.activation(
                        s_sb[:], h1_ps[:], mybir.ActivationFunctionType.Silu
                    )
                nc.vector.tensor_mul(hT[:, it, :], s_sb[:], h3_ps[:])

            for r in range(NI - 4, NI):
                nc.sync.dma_start(w2_sb[:, r, :], w2t_r[:, r, :])

            ps1_pool.release()
            psw_pool.release()

            # Layer 2, t-outer: each 128-token tile accumulates its full
            # 1024-dim output (2 PSUM banks) across all 32 i-tiles, then
            # drains while the next tile accumulates. Gate applied as a
            # per-partition scale on the PSUM->SBUF copy; ACT takes one bank,
            # DVE the other, so the two drains run in parallel.
            ps2_pool = tc.alloc_tile_pool(name="ps2", bufs=2, space="PSUM")
            for tt in range(NT):
                y_ps = ps2_pool.tile([128, 2, 512], F32, tag="y")
                for dc in range(2):
                    for it in range(NI):
                        nc.tensor.matmul(
                            y_ps[:, dc, :],
                            hT[:, it, tt * 128:(tt + 1) * 128],
                            w2_sb[:, it, dc * 512:(dc + 1) * 512],
                            start=(it == 0),
                            stop=(it == NI - 1),
                        )
                    y_sb = out_pool.tile([128, 512], F32, tag="ysb")
                    src = y_ps[:, dc, :]
                    if tt == NT - 1:
                        # Tail-critical: split the final drains across both
                        # engines and two output queues each.
                        for h in range(2):
                            dst = y_sb[:, h * 256:(h + 1) * 256]
                            s2 = y_ps[:, dc, h * 256:(h + 1) * 256]
                            if h == 0:
                                nc.scalar.activation(
                                    dst, s2, mybir.ActivationFunctionType.Copy,
                                    scale=g_sb[:, tt:tt + 1],
                                )
                            else:
                                nc.vector.tensor_scalar_mul(
                                    dst, s2, g_sb[:, tt:tt + 1]
                                )
                            nc.sync.dma_start(
                                y_d[tt * 128:(tt + 1) * 128,
                                    dc * 512 + h * 256:dc * 512 + (h + 1) * 256],
                                dst,
                            )
                    else:
                        if dc == 0:
                            nc.scalar.activation(
                                y_sb[:], src, mybir.ActivationFunctionType.Copy,
                                scale=g_sb[:, tt:tt + 1],
                            )
                        else:
                            nc.vector.tensor_scalar_mul(
                                y_sb[:], src, g_sb[:, tt:tt + 1]
                            )
                        nc.sync.dma_start(
                            y_d[tt * 128:(tt + 1) * 128,
                                dc * 512:(dc + 1) * 512],
                            y_sb[:],
                        )
            ps2_pool.release()

    nc.compile()
    return nc


def _pack_weights(w1, w2, w3):
    """Per-expert device layouts (bf16), all linear HBM reads:
    w1p/w3p[it, p, dt, c] = w[it*128+c, dt*128+p]  (i.e. w.T tiled for lhsT)
    w2t = w2.T ([I, D], i rows on partitions)."""
    key = tuple((a.ctypes.data, a.shape) for a in (w1, w2, w3))
    if _WCACHE.get("key") == key:
        return _WCACHE["maps"]
    maps = []
    for e in range(E):
        w13p = np.empty((NI, 2, 128, DT, 128), dtype=BF)
        w13p[:, 0] = w1[e].reshape(NI, 128, DT, 128).transpose(0, 3, 2, 1)
        w13p[:, 1] = w3[e].reshape(NI, 128, DT, 128).transpose(0, 3, 2, 1)
        w2t = np.ascontiguousarray(w2[e].T.astype(BF))
        maps.append({"w13p": w13p, "w2t": w2t})
    _WCACHE["key"] = key
    _WCACHE["maps"] = maps
    return maps


def kernel(x, expert_indices, expert_weights, w1, w2, w3):
    global _NC, LAST_RESULTS
    x = np.ascontiguousarray(np.asarray(x, dtype=np.float32))
    idx = np.asarray(expert_indices)
    ew = np.asarray(expert_weights, dtype=np.float32)
    w1 = np.ascontiguousarray(np.asarray(w1, dtype=np.float32))
    w2 = np.ascontiguousarray(np.asarray(w2, dtype=np.float32))
    w3 = np.ascontiguousarray(np.asarray(w3, dtype=np.float32))

    if _NC is None:
        _NC = _build_nc()

    # Host routing: unique tokens per expert, with both top-k gate weights of a
    # token merged (a token picking the same expert twice gets the summed gate).
    tok_lists, gate_lists = [], []
    for e in range(E):
        m = idx == e
        sel = np.nonzero(m.any(axis=1))[0]
        tok_lists.append(sel)
        gate_lists.append((ew * m).sum(axis=1)[sel].astype(np.float32))

    weight_maps = _pack_weights(w1, w2, w3)
    x_bf = x.astype(BF)

    n_pass = max(1, math.ceil(max(len(s) for s in tok_lists) / CAP))
    out = np.zeros((T, D), dtype=np.float32)
    trace = bool(os.environ.get("BASS_TRACE"))
    for p in range(n_pass):
        in_maps = []
        chunks = []
        for e in range(E):
            sel = tok_lists[e][p * CAP:(p + 1) * CAP]
            g = gate_lists[e][p * CAP:(p + 1) * CAP]
            chunks.append(sel)
            xt = np.zeros((DT, 128, CAP), dtype=BF)
            if len(sel):
                xt.reshape(D, CAP)[:, :len(sel)] = x_bf[sel].T
            g_pad = np.zeros((CAP,), dtype=np.float32)
            g_pad[:len(sel)] = g
            in_maps.append({"xt": xt, "g": g_pad, **weight_maps[e]})
        # Rare transient NRT_EXEC_UNIT_UNRECOVERABLE errors have been observed
        # on the first execution of a fresh NEFF; a straight retry recovers.
        last_exc = None
        for attempt in range(3):
            try:
                LAST_RESULTS = run_bass_kernel_spmd(
                    _NC, in_maps, core_ids=list(range(N_CORES)),
                    trace=trace and attempt == 0,
                )
                break
            except Exception as exc:  # noqa: BLE001
                last_exc = exc
                time.sleep(3)
        else:
            raise last_exc
        for e in range(E):
            sel = chunks[e]
            if len(sel):
                out[sel] += LAST_RESULTS.results[e]["y"][:len(sel)]
    return out
